# revision 4
# baseline (speedup 1.0000x reference)
"""Trainium2 Bass kernel for the 2D-LSTM (nn_Lstm2D) problem.

Reference computation (B=64, C=3, H=W=128, P=4 patch, NC=512 cells):
  - image is cut into a 32x32 grid of 4x4 patches, raster-scanned (1024 steps)
  - per step t=(i,j):  gates = [x_t, h_prevrow_j] @ W_ih.T + h_{t-1} @ W_hh.T + b
                       i,f,g,o = split(gates); c = sig(f)*c + sig(i)*tanh(g)
                       h = sig(o)*tanh(c)
  - output: h at every grid cell -> (B, 512, 32, 32)

Strategy (8 NeuronCores, data-parallel over batch, 8 batch elements/core):
  The recurrence is a strict 1024-step serial chain; per step the TensorE
  must reload 64 weight tiles (16 gate-chunks x 4 k-chunks, ~30ns each) and
  the sigmoid/tanh/vector tail adds ~1.6us of cross-engine latency.  A
  single chain is latency-bound (~3.4us/step).  So each core runs TWO
  independent chains (batch split 4+4): while chain A's activation tail is
  in flight, the PE executes chain B's sweep.  Step period becomes PE-work
  bound (~2x64 matmuls + PRE per pair).

  - per-row "PRE" (bias + x@Wx.T + prevrow@Wp.T) accumulates in PSUM over
    2 bands of 16 steps; bias rides as a ones-row of x.  Recurrence matmuls
    accumulate on top (start=False), one PRE slot interleaved per step.
  - gate PSUM: per chain per band-set one [128,16slots,16steps,4batch] f32
    region (2 banks); 2 chains x 2 sets = 8 banks.  ONE sigmoid covers all
    16 slots (g-gate rows pre-scaled x2: tanh(g)=2*sig(2g)-1).
  - sweep matmuls write a same-address PSUM alias (no sem updates) except
    the last, whose auto-dep gates the sigmoid (avoids the serialized
    sem-inc conveyor).
  - chain tail: sig(all) -> fc=sig(f)*c, t2=(sig2g-.5)*sig(i), c=2*t2+fc
    (DVE) -> tanh(c) (ACT) -> h = tanh_c*sig(o) (DVE, bf16 row buffer).
"""

import numpy as np
import ml_dtypes

B = 64
C = 3
H = W = 128
P = 4
NCELL = 512
IN = C * P * P           # 48
IN1 = IN + 1             # +1 ones-row carrying the bias
KPAD = 128               # x contraction zero-padded to full partition dim
SY = SX = 32
NCORES = 8
BL = B // NCORES         # 8 batch elements per core
NCHAIN = 2
BL2 = BL // NCHAIN       # 4 batch elements per chain
KC = NCELL // 128        # 4 contraction chunks for h
MC = (4 * NCELL) // 128  # 16 gate-dim chunks
NBAND = 2                # 2 bands of 16 steps per row
BSTEP = SX // NBAND      # 16 steps per band
# slot order (m-chunks of 128 gate rows): f0..f3, g0..g3, i0..i3, o0..o3
SLOT_TO_MCHUNK = [4, 5, 6, 7, 8, 9, 10, 11, 0, 1, 2, 3, 12, 13, 14, 15]

BF16 = ml_dtypes.bfloat16


def _build_module(sy=SY):
    import concourse.bass as bass
    import concourse.bacc as bacc
    import concourse.tile as tile
    import concourse.mybir as mybir

    f32 = mybir.dt.float32
    bf16 = mybir.dt.bfloat16
    SIG = mybir.ActivationFunctionType.Sigmoid
    TANH = mybir.ActivationFunctionType.Tanh
    SUB = mybir.AluOpType.subtract
    MULT = mybir.AluOpType.mult
    ADD = mybir.AluOpType.add

    nc = bacc.Bacc()

    # x rows padded by one (prefetch of row sy reads harmless zeros)
    x_d = nc.declare_dram_parameter("xt", [KPAD, sy + 1, NCHAIN, SX * BL2],
                                    bf16, isOutput=False)
    whh_d = nc.declare_dram_parameter("whht", [128, KC * MC * 128], bf16,
                                      isOutput=False)
    wp_d = nc.declare_dram_parameter("wpt", [128, KC * MC * 128], bf16,
                                     isOutput=False)
    wx_d = nc.declare_dram_parameter("wxt", [KPAD, MC * 128], bf16,
                                     isOutput=False)
    out_d = [nc.declare_dram_parameter(f"out{cch}", [128, KC, sy * SX, BL2],
                                       bf16, isOutput=True)
             for cch in range(NCHAIN)]

    with tile.TileContext(nc) as tc:
        with (
            tc.tile_pool(name="persist", bufs=1) as persist,
            tc.tile_pool(name="acts", bufs=4) as actspool,
            tc.tile_pool(name="tmp", bufs=8) as tpool,
        ):
            whh_sb = persist.tile([128, KC, MC, 128], bf16)
            wp_sb = persist.tile([128, KC, MC, 128], bf16)
            wx_sb = persist.tile([KPAD, MC, 128], bf16)
            c_sb = [persist.tile([128, KC, BL2], f32, name=f"c_sb{i}")
                    for i in range(NCHAIN)]
            # h row buffers, ping-pong by row parity, per chain
            hbuf = [[persist.tile([128, KC, SX, BL2], bf16, name=f"hb{i}{p}")
                     for p in range(2)] for i in range(NCHAIN)]
            xr = [[persist.tile([KPAD, SX * BL2], bf16, name=f"xr{i}{p}")
                   for p in range(2)] for i in range(NCHAIN)]

            # gate PSUM: [slots 16, steps 16, batch 4] f32 = 4KB = 2 banks.
            # chain c, band-set s -> banks 4c+2s..4c+2s+1.  The read alias
            # (same address) lets all but the last sweep matmul skip sem
            # updates; the last one's auto-dep gates the sigmoid.
            PS = [[nc.place_psum_tensor(f"p{cc}{ss}", [128, MC, BSTEP, BL2],
                                        f32, 4 * cc + 2 * ss)
                   for ss in range(2)] for cc in range(NCHAIN)]
            PA = [[nc.place_psum_tensor(f"pa{cc}{ss}", [128, MC, BSTEP, BL2],
                                        f32, 4 * cc + 2 * ss)
                   for ss in range(2)] for cc in range(NCHAIN)]

            nc.sync.dma_start(out=whh_sb[:], in_=whh_d[:])
            nc.sync.dma_start(out=wp_sb[:], in_=wp_d[:])
            nc.sync.dma_start(out=wx_sb[:], in_=wx_d[:])
            for cc in range(NCHAIN):
                nc.vector.memset(c_sb[cc][:], 0.0)
                nc.vector.memset(hbuf[cc][0][:], 0.0)
                nc.vector.memset(hbuf[cc][1][:], 0.0)

            # pull the sigmoid/tanh ACT table load out of the row loop
            warm = persist.tile([1, 1], f32)
            nc.vector.memset(warm[:], 0.0)
            nc.scalar.activation(out=warm[:], in_=warm[:], func=SIG)
            nc.scalar.activation(out=warm[:], in_=warm[:], func=TANH)

            def pre_slot(cc, s, band, band_set, xrt, hsrc):
                # bias + x @ Wx.T + prevrow @ Wp.T for 16 steps of one slot.
                # start=True only on the first slot of each 2KB PSUM bank
                # (slots 0-7 bank A, 8-15 bank B): the start flag invalidates
                # the whole bank region, so per-slot start would wipe
                # previously accumulated slots.
                dst = PS[cc][band_set][:, s, :, :]
                nc.tensor.matmul(
                    dst, wx_sb[:, s, :],
                    xrt[:, band * BSTEP * BL2:(band + 1) * BSTEP * BL2],
                    start=s in (0, 8), stop=False, skip_group_check=True)
                for k in range(KC):
                    nc.tensor.matmul(
                        dst, wp_sb[:, k, s, :],
                        hsrc[:, k, band * BSTEP:(band + 1) * BSTEP, :],
                        start=False, stop=False, skip_group_check=True)

            def emit_chain_step(cc, j, cur, prev):
                band_set = (j // BSTEP) % 2
                jl = j % BSTEP

                def rhs_h(k):
                    if j == 0:
                        return prev[:, k, SX - 1, :]
                    return cur[:, k, j - 1, :]

                # sweep: 64 matmuls; only the last writes the tracked tensor
                for s in range(MC):
                    for k in range(KC):
                        last = s == MC - 1 and k == KC - 1
                        dst = (PS[cc][band_set][:, s, jl, :] if last
                               else PA[cc][band_set][:, s, jl, :])
                        nc.tensor.matmul(
                            dst, whh_sb[:, k, s, :], rhs_h(k),
                            start=False, stop=(k == KC - 1),
                            skip_group_check=True)

                # chain tail
                acts = actspool.tile([128, MC, BL2], f32)
                nc.scalar.activation(
                    out=acts[:], in_=PS[cc][band_set][:, :, jl, :], func=SIG)
                fc = tpool.tile([128, KC, BL2], f32)
                nc.vector.tensor_mul(fc[:], acts[:, 0:4, :], c_sb[cc][:])
                t2 = tpool.tile([128, KC, BL2], f32)
                nc.vector.scalar_tensor_tensor(
                    out=t2[:], in0=acts[:, 4:8, :], scalar=0.5,
                    in1=acts[:, 8:12, :], op0=SUB, op1=MULT)
                nc.vector.scalar_tensor_tensor(
                    out=c_sb[cc][:], in0=t2[:], scalar=2.0, in1=fc[:],
                    op0=MULT, op1=ADD)
                tc_t = tpool.tile([128, KC, BL2], f32)
                nc.scalar.activation(out=tc_t[:], in_=c_sb[cc][:], func=TANH)
                nc.vector.tensor_mul(cur[:, :, j, :], acts[:, 12:16, :],
                                     tc_t[:])

            def emit_pre_for_step(cc, j, cur, prev, xr_cur, xr_nxt):
                # during band b emit band b+1's PRE, one slot per step
                # (skip jl=0: its WAR on the band-set may not be clear yet)
                band_next = j // BSTEP + 1
                jl = j % BSTEP
                if jl == 0:
                    return
                slots = [jl - 1] + ([15] if jl == BSTEP - 1 else [])
                for s in slots:
                    if band_next < NBAND:
                        pre_slot(cc, s, band_next, band_next % 2, xr_cur,
                                 prev)
                    else:  # next row's band 0 (uses this row's h cols 0..15)
                        pre_slot(cc, s, 0, 0, xr_nxt, cur)

            def row_section(par, row_expr):
                nxt = 1 - par
                # prefetch next row's x (row sy reads the zero padding)
                for cc in range(NCHAIN):
                    nc.gpsimd.dma_start(
                        out=xr[cc][nxt][:],
                        in_=x_d[:, bass.ds(row_expr + 1, 1), cc, :])
                for j in range(SX):
                    for cc in range(NCHAIN):
                        emit_chain_step(cc, j, hbuf[cc][par], hbuf[cc][nxt])
                    for cc in range(NCHAIN):
                        emit_pre_for_step(cc, j, hbuf[cc][par],
                                          hbuf[cc][nxt], xr[cc][par],
                                          xr[cc][nxt])
                for cc in range(NCHAIN):
                    nc.gpsimd.dma_start(
                        out=out_d[cc][:, :, bass.ds(row_expr * SX, SX), :],
                        in_=hbuf[cc][par][:])

            # row 0: x + band 0
            for cc in range(NCHAIN):
                nc.gpsimd.dma_start(out=xr[cc][0][:], in_=x_d[:, 0, cc, :])
            for cc in range(NCHAIN):
                for s in range(MC):
                    pre_slot(cc, s, 0, 0, xr[cc][0], hbuf[cc][1])

            with tc.For_i(0, sy // 2) as iv:
                row_section(0, iv * 2)
                row_section(1, iv * 2 + 1)

    nc.compile()
    return nc


_CACHE = {}


def _get_module(sy=SY):
    if sy not in _CACHE:
        _CACHE[sy] = _build_module(sy)
    return _CACHE[sy]


def _prep_shared(W_ih, W_hh, b_ih, b_hh):
    perm = np.array(SLOT_TO_MCHUNK)
    scale = np.ones((16, 1), np.float32)
    scale[8:12] = 2.0  # g-gate rows pre-scaled: tanh(g) = 2*sig(2g) - 1

    wih_t = np.ascontiguousarray(W_ih.T.astype(np.float32))     # (560, 2048)
    wih_t = (wih_t.reshape(560, 16, 128) * scale[None]).astype(np.float32)
    wih_t = wih_t[:, perm, :]                                   # slot order
    bias = ((b_ih + b_hh).astype(np.float32).reshape(16, 128) * scale)[perm]
    wx = np.zeros((KPAD, 16, 128), np.float32)
    wx[:IN] = wih_t[:IN]
    wx[IN] = bias
    wx = wx.reshape(KPAD, MC * 128)
    wp = wih_t[IN:]                                             # (512,16,128)
    wp = wp.reshape(KC, 128, MC, 128).transpose(1, 0, 2, 3)
    wp = wp.reshape(128, KC * MC * 128)
    whh = np.ascontiguousarray(W_hh.T.astype(np.float32))       # (512, 2048)
    whh = (whh.reshape(512, 16, 128) * scale[None])[:, perm, :]
    whh = whh.reshape(KC, 128, MC, 128).transpose(1, 0, 2, 3)
    whh = whh.reshape(128, KC * MC * 128)
    return wx.astype(BF16), wp.astype(BF16), whh.astype(BF16)


def _prep_x(batch, sy=SY):
    # xs[i, j, b, :] = patch (C,P,P) flattened, matching the reference
    xs = batch.reshape(B, C, sy, P, SX, P).transpose(2, 4, 0, 1, 3, 5)
    xs = xs.reshape(sy, SX, B, IN)
    per_core = []
    for core in range(NCORES):
        xa = np.zeros((KPAD, sy + 1, NCHAIN, SX * BL2), np.float32)
        for cc in range(NCHAIN):
            b0 = core * BL + cc * BL2
            xc = xs[:, :, b0:b0 + BL2, :]              # (sy, SX, BL2, IN)
            xc = xc.transpose(3, 0, 1, 2).reshape(IN, sy, SX * BL2)
            xa[:IN, :sy, cc] = xc
        xa[IN, :, :, :] = 1.0                          # bias ones-row
        per_core.append(xa.astype(BF16))
    return per_core


def _run(batch, W_ih, W_hh, b_ih, b_hh, trace=False):
    from concourse.bass_utils import run_bass_kernel_spmd

    batch = np.asarray(batch, dtype=np.float32)
    wx, wp, whh = _prep_shared(
        np.asarray(W_ih), np.asarray(W_hh), np.asarray(b_ih), np.asarray(b_hh))
    xs = _prep_x(batch)

    nc = _get_module()
    in_maps = [
        {"xt": xs[c], "whht": whh, "wpt": wp, "wxt": wx}
        for c in range(NCORES)
    ]
    res = run_bass_kernel_spmd(nc, in_maps, list(range(NCORES)), trace=trace)

    outs = []
    for core in range(NCORES):
        for cc in range(NCHAIN):
            arr = res.results[core][f"out{cc}"].astype(np.float32)
            # (128, KC, T, BL2) -> (BL2, T, KC, 128) -> (BL2, NC, SY, SX)
            arr = arr.transpose(3, 2, 1, 0).reshape(BL2, NCELL, SY, SX)
            outs.append(arr)
    return np.concatenate(outs, axis=0).astype(np.float32), res


def kernel(batch, W_ih, W_hh, b_ih, b_hh):
    out, _ = _run(batch, W_ih, W_hh, b_ih, b_hh)
    return out


# revision 8
# speedup vs baseline: 1.4440x; 1.4440x over previous
"""Trainium2 Bass kernel for the 2D-LSTM (nn_Lstm2D) problem.

Reference computation (B=64, C=3, H=W=128, P=4 patch, NC=512 cells):
  - image is cut into a 32x32 grid of 4x4 patches, raster-scanned (1024 steps)
  - per step t=(i,j):  gates = [x_t, h_prevrow_j] @ W_ih.T + h_{t-1} @ W_hh.T + b
                       i,f,g,o = split(gates); c = sig(f)*c + sig(i)*tanh(g)
                       h = sig(o)*tanh(c)
  - output: h at every grid cell -> (B, 512, 32, 32)

Strategy (8 NeuronCores, data-parallel over batch, 8 batch elements/core):
  The recurrence is a strict 1024-step serial chain; per step the TensorE
  must reload 64 weight tiles (16 gate-chunks x 4 k-chunks, ~30ns each) and
  the sigmoid/tanh/vector tail adds ~1.6us of cross-engine latency.  A
  single chain is latency-bound (~3.4us/step).  So each core runs TWO
  independent chains (batch split 4+4): while chain A's activation tail is
  in flight, the PE executes chain B's sweep.  Step period becomes PE-work
  bound (~2x64 matmuls + PRE per pair).

  - per-row "PRE" (bias + x@Wx.T + prevrow@Wp.T) accumulates in PSUM over
    2 bands of 16 steps; bias rides as a ones-row of x.  Recurrence matmuls
    accumulate on top (start=False), one PRE slot interleaved per step.
  - gate PSUM: per chain per band-set one [128,16slots,16steps,4batch] f32
    region (2 banks); 2 chains x 2 sets = 8 banks.  ONE sigmoid covers all
    16 slots (g-gate rows pre-scaled x2: tanh(g)=2*sig(2g)-1).
  - sweep matmuls write a same-address PSUM alias (no sem updates) except
    the last, whose auto-dep gates the sigmoid (avoids the serialized
    sem-inc conveyor).
  - chain tail: sig(all) -> fc=sig(f)*c, t2=(sig2g-.5)*sig(i), c=2*t2+fc
    (DVE) -> tanh(c) (ACT) -> h = tanh_c*sig(o) (DVE, bf16 row buffer).
"""

import numpy as np
import ml_dtypes

B = 64
C = 3
H = W = 128
P = 4
NCELL = 512
IN = C * P * P           # 48
IN1 = IN + 1             # +1 ones-row carrying the bias
KPAD = 128               # x contraction zero-padded to full partition dim
SY = SX = 32
NCORES = 8
BL = B // NCORES         # 8 batch elements per core
NCHAIN = 2
BL2 = BL // NCHAIN       # 4 batch elements per chain
KC = NCELL // 128        # 4 contraction chunks for h
MC = (4 * NCELL) // 128  # 16 gate-dim chunks
NBAND = 2                # 2 bands of 16 steps per row
BSTEP = SX // NBAND      # 16 steps per band
# slot order (m-chunks of 128 gate rows): f0..f3, g0..g3, i0..i3, o0..o3
SLOT_TO_MCHUNK = [4, 5, 6, 7, 8, 9, 10, 11, 0, 1, 2, 3, 12, 13, 14, 15]

BF16 = ml_dtypes.bfloat16


def _build_module(sy=SY):
    import concourse.bass as bass
    import concourse.bacc as bacc
    import concourse.tile as tile
    import concourse.mybir as mybir

    f32 = mybir.dt.float32
    bf16 = mybir.dt.bfloat16
    SIG = mybir.ActivationFunctionType.Sigmoid
    TANH = mybir.ActivationFunctionType.Tanh
    SUB = mybir.AluOpType.subtract
    MULT = mybir.AluOpType.mult
    ADD = mybir.AluOpType.add

    nc = bacc.Bacc()

    # x rows padded by one (prefetch of row sy reads harmless zeros)
    x_d = nc.declare_dram_parameter("xt", [KPAD, sy + 1, NCHAIN, SX * BL2],
                                    bf16, isOutput=False)
    whh_d = nc.declare_dram_parameter("whht", [128, KC * MC * 128], bf16,
                                      isOutput=False)
    wp_d = nc.declare_dram_parameter("wpt", [128, KC * MC * 128], bf16,
                                     isOutput=False)
    wx_d = nc.declare_dram_parameter("wxt", [KPAD, MC * 128], bf16,
                                     isOutput=False)
    out_d = [nc.declare_dram_parameter(f"out{cch}", [128, KC, sy * SX, BL2],
                                       bf16, isOutput=True)
             for cch in range(NCHAIN)]

    with tile.TileContext(nc) as tc:
        with (
            tc.tile_pool(name="persist", bufs=1) as persist,
            tc.tile_pool(name="acts", bufs=4) as actspool,
            tc.tile_pool(name="tmp", bufs=8) as tpool,
        ):
            whh_sb = persist.tile([128, KC, MC, 128], bf16)
            wp_sb = persist.tile([128, KC, MC, 128], bf16)
            wx_sb = persist.tile([KPAD, MC, 128], bf16)
            c_sb = [persist.tile([128, KC, BL2], f32, name=f"c_sb{i}")
                    for i in range(NCHAIN)]
            # h row buffers, ping-pong by row parity, per chain
            hbuf = [[persist.tile([128, KC, SX, BL2], bf16, name=f"hb{i}{p}")
                     for p in range(2)] for i in range(NCHAIN)]
            xr = [[persist.tile([KPAD, SX * BL2], bf16, name=f"xr{i}{p}")
                   for p in range(2)] for i in range(NCHAIN)]

            # gate PSUM: [slots 16, steps 16, batch 4] f32 = 4KB = 2 banks.
            # chain c, band-set s -> banks 4c+2s..4c+2s+1.  The read alias
            # (same address) lets all but the last sweep matmul skip sem
            # updates; the last one's auto-dep gates the sigmoid.
            PS = [[nc.place_psum_tensor(f"p{cc}{ss}", [128, MC, BSTEP, BL2],
                                        f32, 4 * cc + 2 * ss)
                   for ss in range(2)] for cc in range(NCHAIN)]
            PA = [[nc.place_psum_tensor(f"pa{cc}{ss}", [128, MC, BSTEP, BL2],
                                        f32, 4 * cc + 2 * ss)
                   for ss in range(2)] for cc in range(NCHAIN)]

            nc.sync.dma_start(out=whh_sb[:], in_=whh_d[:])
            nc.sync.dma_start(out=wp_sb[:], in_=wp_d[:])
            nc.sync.dma_start(out=wx_sb[:], in_=wx_d[:])
            for cc in range(NCHAIN):
                nc.vector.memset(c_sb[cc][:], 0.0)
                nc.vector.memset(hbuf[cc][0][:], 0.0)
                nc.vector.memset(hbuf[cc][1][:], 0.0)

            # pull the sigmoid/tanh ACT table load out of the row loop
            warm = persist.tile([1, 1], f32)
            nc.vector.memset(warm[:], 0.0)
            nc.scalar.activation(out=warm[:], in_=warm[:], func=SIG)
            nc.scalar.activation(out=warm[:], in_=warm[:], func=TANH)

            def pre_slot(cc, s, band, band_set, xrt, hsrc):
                # bias + x @ Wx.T + prevrow @ Wp.T for 16 steps of one slot.
                # start=True only on the first slot of each 2KB PSUM bank
                # (slots 0-7 bank A, 8-15 bank B): the start flag invalidates
                # the whole bank region, so per-slot start would wipe
                # previously accumulated slots.
                dst = PS[cc][band_set][:, s, :, :]
                nc.tensor.matmul(
                    dst, wx_sb[:, s, :],
                    xrt[:, band * BSTEP * BL2:(band + 1) * BSTEP * BL2],
                    start=s in (0, 8), stop=False, skip_group_check=True)
                for k in range(KC):
                    nc.tensor.matmul(
                        dst, wp_sb[:, k, s, :],
                        hsrc[:, k, band * BSTEP:(band + 1) * BSTEP, :],
                        start=False, stop=False, skip_group_check=True)

            # manual scheduling clock: each pair-step gets its own sim-time
            # window so the static scheduler keeps our per-engine stream
            # order (its cost model mis-times matmuls and would otherwise
            # reorder ACT/DVE ops and bunch the PRE matmuls).
            ts_clock = [1.0]

            def emit_sweep(cc, j, cur, prev):
                band_set = (j // BSTEP) % 2
                jl = j % BSTEP

                def rhs_h(k):
                    if j == 0:
                        return prev[:, k, SX - 1, :]
                    return cur[:, k, j - 1, :]

                # sweep: 64 matmuls; only the last writes the tracked tensor
                for s in range(MC):
                    for k in range(KC):
                        last = s == MC - 1 and k == KC - 1
                        dst = (PS[cc][band_set][:, s, jl, :] if last
                               else PA[cc][band_set][:, s, jl, :])
                        nc.tensor.matmul(
                            dst, whh_sb[:, k, s, :], rhs_h(k),
                            start=False, stop=(k == KC - 1),
                            skip_group_check=True)

            def emit_tail(cc, j, cur, ts):
                band_set = (j // BSTEP) % 2
                jl = j % BSTEP
                with tc.tile_wait_until(ts + 0.001):
                    acts = actspool.tile([128, MC, BL2], f32)
                    nc.scalar.activation(
                        out=acts[:], in_=PS[cc][band_set][:, :, jl, :],
                        func=SIG)
                with tc.tile_wait_until(ts + 0.002):
                    fc = tpool.tile([128, KC, BL2], f32)
                    nc.vector.tensor_mul(fc[:], acts[:, 0:4, :], c_sb[cc][:])
                    t2 = tpool.tile([128, KC, BL2], f32)
                    nc.vector.scalar_tensor_tensor(
                        out=t2[:], in0=acts[:, 4:8, :], scalar=0.5,
                        in1=acts[:, 8:12, :], op0=SUB, op1=MULT)
                    nc.vector.scalar_tensor_tensor(
                        out=c_sb[cc][:], in0=t2[:], scalar=2.0, in1=fc[:],
                        op0=MULT, op1=ADD)
                with tc.tile_wait_until(ts + 0.003):
                    tc_t = tpool.tile([128, KC, BL2], f32)
                    nc.scalar.activation(out=tc_t[:], in_=c_sb[cc][:],
                                         func=TANH)
                with tc.tile_wait_until(ts + 0.004):
                    nc.vector.tensor_mul(cur[:, :, j, :], acts[:, 12:16, :],
                                         tc_t[:])

            def emit_pre_for_step(cc, j, cur, prev, xr_cur, xr_nxt):
                # during band b emit band b+1's PRE, one slot per step
                # (skip jl=0: its WAR on the band-set may not be clear yet)
                band_next = j // BSTEP + 1
                jl = j % BSTEP
                if jl == 0:
                    return
                slots = [jl - 1] + ([15] if jl == BSTEP - 1 else [])
                for s in slots:
                    if band_next < NBAND:
                        pre_slot(cc, s, band_next, band_next % 2, xr_cur,
                                 prev)
                    else:  # next row's band 0 (uses this row's h cols 0..15)
                        pre_slot(cc, s, 0, 0, xr_nxt, cur)

            def row_section(par, row_expr):
                nxt = 1 - par
                # prefetch next row's x (row sy reads the zero padding)
                with tc.tile_wait_until(ts_clock[0]):
                    for cc in range(NCHAIN):
                        nc.gpsimd.dma_start(
                            out=xr[cc][nxt][:],
                            in_=x_d[:, bass.ds(row_expr + 1, 1), cc, :])
                for j in range(SX):
                    ts = ts_clock[0]
                    ts_clock[0] += 0.01
                    # PE stream: swA, swB, preA, preB -- all pinned to this
                    # step's window; tails get sub-slots so the ACT stream
                    # stays sigA, tanhA, sigB, tanhB.
                    with tc.tile_wait_until(ts):
                        emit_sweep(0, j, hbuf[0][par], hbuf[0][nxt])
                    emit_tail(0, j, hbuf[0][par], ts)
                    with tc.tile_wait_until(ts + 0.005):
                        emit_sweep(1, j, hbuf[1][par], hbuf[1][nxt])
                    emit_tail(1, j, hbuf[1][par], ts + 0.005)
                    with tc.tile_wait_until(ts + 0.005):
                        for cc in range(NCHAIN):
                            emit_pre_for_step(cc, j, hbuf[cc][par],
                                              hbuf[cc][nxt], xr[cc][par],
                                              xr[cc][nxt])
                with tc.tile_wait_until(ts_clock[0]):
                    for cc in range(NCHAIN):
                        nc.gpsimd.dma_start(
                            out=out_d[cc][:, :,
                                          bass.ds(row_expr * SX, SX), :],
                            in_=hbuf[cc][par][:])

            # row 0: x + band 0
            for cc in range(NCHAIN):
                nc.gpsimd.dma_start(out=xr[cc][0][:], in_=x_d[:, 0, cc, :])
            for cc in range(NCHAIN):
                for s in range(MC):
                    pre_slot(cc, s, 0, 0, xr[cc][0], hbuf[cc][1])

            with tc.For_i(0, sy // 4) as iv:
                for rr in range(4):
                    row_section(rr % 2, iv * 4 + rr)

    nc.compile()
    return nc


_CACHE = {}


def _get_module(sy=SY):
    if sy not in _CACHE:
        _CACHE[sy] = _build_module(sy)
    return _CACHE[sy]


def _prep_shared(W_ih, W_hh, b_ih, b_hh):
    perm = np.array(SLOT_TO_MCHUNK)
    scale = np.ones((16, 1), np.float32)
    scale[8:12] = 2.0  # g-gate rows pre-scaled: tanh(g) = 2*sig(2g) - 1

    wih_t = np.ascontiguousarray(W_ih.T.astype(np.float32))     # (560, 2048)
    wih_t = (wih_t.reshape(560, 16, 128) * scale[None]).astype(np.float32)
    wih_t = wih_t[:, perm, :]                                   # slot order
    bias = ((b_ih + b_hh).astype(np.float32).reshape(16, 128) * scale)[perm]
    wx = np.zeros((KPAD, 16, 128), np.float32)
    wx[:IN] = wih_t[:IN]
    wx[IN] = bias
    wx = wx.reshape(KPAD, MC * 128)
    wp = wih_t[IN:]                                             # (512,16,128)
    wp = wp.reshape(KC, 128, MC, 128).transpose(1, 0, 2, 3)
    wp = wp.reshape(128, KC * MC * 128)
    whh = np.ascontiguousarray(W_hh.T.astype(np.float32))       # (512, 2048)
    whh = (whh.reshape(512, 16, 128) * scale[None])[:, perm, :]
    whh = whh.reshape(KC, 128, MC, 128).transpose(1, 0, 2, 3)
    whh = whh.reshape(128, KC * MC * 128)
    return wx.astype(BF16), wp.astype(BF16), whh.astype(BF16)


def _prep_x(batch, sy=SY):
    # xs[i, j, b, :] = patch (C,P,P) flattened, matching the reference
    xs = batch.reshape(B, C, sy, P, SX, P).transpose(2, 4, 0, 1, 3, 5)
    xs = xs.reshape(sy, SX, B, IN)
    per_core = []
    for core in range(NCORES):
        xa = np.zeros((KPAD, sy + 1, NCHAIN, SX * BL2), np.float32)
        for cc in range(NCHAIN):
            b0 = core * BL + cc * BL2
            xc = xs[:, :, b0:b0 + BL2, :]              # (sy, SX, BL2, IN)
            xc = xc.transpose(3, 0, 1, 2).reshape(IN, sy, SX * BL2)
            xa[:IN, :sy, cc] = xc
        xa[IN, :, :, :] = 1.0                          # bias ones-row
        per_core.append(xa.astype(BF16))
    return per_core


def _run(batch, W_ih, W_hh, b_ih, b_hh, trace=False):
    from concourse.bass_utils import run_bass_kernel_spmd

    batch = np.asarray(batch, dtype=np.float32)
    wx, wp, whh = _prep_shared(
        np.asarray(W_ih), np.asarray(W_hh), np.asarray(b_ih), np.asarray(b_hh))
    xs = _prep_x(batch)

    nc = _get_module()
    in_maps = [
        {"xt": xs[c], "whht": whh, "wpt": wp, "wxt": wx}
        for c in range(NCORES)
    ]
    res = run_bass_kernel_spmd(nc, in_maps, list(range(NCORES)), trace=trace)

    outs = []
    for core in range(NCORES):
        for cc in range(NCHAIN):
            arr = res.results[core][f"out{cc}"].astype(np.float32)
            # (128, KC, T, BL2) -> (BL2, T, KC, 128) -> (BL2, NC, SY, SX)
            arr = arr.transpose(3, 2, 1, 0).reshape(BL2, NCELL, SY, SX)
            outs.append(arr)
    return np.concatenate(outs, axis=0).astype(np.float32), res


def kernel(batch, W_ih, W_hh, b_ih, b_hh):
    out, _ = _run(batch, W_ih, W_hh, b_ih, b_hh)
    return out


# revision 9
# speedup vs baseline: 1.8104x; 1.2538x over previous
"""Trainium2 Bass kernel for the 2D-LSTM (nn_Lstm2D) problem.

Reference computation (B=64, C=3, H=W=128, P=4 patch, NC=512 cells):
  - image is cut into a 32x32 grid of 4x4 patches, raster-scanned (1024 steps)
  - per step t=(i,j):  gates = [x_t, h_prevrow_j] @ W_ih.T + h_{t-1} @ W_hh.T + b
                       i,f,g,o = split(gates); c = sig(f)*c + sig(i)*tanh(g)
                       h = sig(o)*tanh(c)
  - output: h at every grid cell -> (B, 512, 32, 32)

Strategy (8 NeuronCores, data-parallel over batch, 8 batch elements/core):
  - The recurrence is a strict 1024-step serial chain; per step the PE runs
    64 weight-tile matmuls (16 gate-chunks x 4 k-chunks, ~34ns each, weight
    load bound) and the sigmoid/tanh/vector tail trails the sweep.
  - per-row "PRE" (bias + x@Wx.T + prevrow@Wp.T) accumulates in PSUM over
    2 bands of 16 steps (N=128 moving -> only 5 PRE matmuls per step); the
    bias rides as a ones-row of x.  Recurrence matmuls accumulate on top
    (start=False); start=True only on the first slot of each 2KB bank.
  - gate PSUM per band-set: [16 slots, 16 steps, 8] f32 = 4 banks; slots
    f 0-3 / g 4-7 / i 8-11 / o 12-15, so sig(f) and sig(gi) fire mid-sweep
    and only sig(o)/tanh/h-mul trail the last matmul.  g-gate rows are
    pre-scaled x2 host-side: tanh(g) = 2*sig(2g)-1.
  - sweep matmuls write a same-address PSUM alias (no tracked deps) except
    each gate group's last, whose auto-dep gates that group's sigmoid.
  - the static scheduler mis-times matmuls and would reorder the ACT/DVE
    streams and bunch PRE matmuls; tile_wait_until sim-time slots pin the
    per-engine stream order (PE: sweep then PRE chunk each step; ACT:
    sig_f, sig_gi, sig_o, tanh; DVE: fc, t2, c, h).
"""

import numpy as np
import ml_dtypes

B = 64
C = 3
H = W = 128
P = 4
NCELL = 512
IN = C * P * P           # 48
IN1 = IN + 1             # +1 ones-row carrying the bias
KPAD = 128               # x contraction zero-padded to full partition dim
SY = SX = 32
NCORES = 8
BL = B // NCORES         # 8 batch elements per core
KC = NCELL // 128        # 4 contraction chunks for h
MC = (4 * NCELL) // 128  # 16 gate-dim chunks
NBAND = 2                # 2 bands of 16 steps per row
BSTEP = SX // NBAND      # 16 steps per band
UNROLL = 8               # rows per hardware-loop iteration
# slot order (m-chunks of 128 gate rows): f0..f3, g0..g3, i0..i3, o0..o3
SLOT_TO_MCHUNK = [4, 5, 6, 7, 8, 9, 10, 11, 0, 1, 2, 3, 12, 13, 14, 15]

BF16 = ml_dtypes.bfloat16


def _build_module(sy=SY):
    import concourse.bass as bass
    import concourse.bacc as bacc
    import concourse.tile as tile
    import concourse.mybir as mybir

    f32 = mybir.dt.float32
    bf16 = mybir.dt.bfloat16
    SIG = mybir.ActivationFunctionType.Sigmoid
    TANH = mybir.ActivationFunctionType.Tanh
    SUB = mybir.AluOpType.subtract
    MULT = mybir.AluOpType.mult
    ADD = mybir.AluOpType.add

    nc = bacc.Bacc()

    # x rows padded by one (prefetch of row sy reads harmless zeros)
    x_d = nc.declare_dram_parameter("xt", [KPAD, sy + 1, SX * BL], bf16,
                                    isOutput=False)
    whh_d = nc.declare_dram_parameter("whht", [128, KC * MC * 128], bf16,
                                      isOutput=False)
    wp_d = nc.declare_dram_parameter("wpt", [128, KC * MC * 128], bf16,
                                     isOutput=False)
    wx_d = nc.declare_dram_parameter("wxt", [KPAD, MC * 128], bf16,
                                     isOutput=False)
    out_d = nc.declare_dram_parameter("out", [128, KC, sy * SX, BL], bf16,
                                      isOutput=True)

    with tile.TileContext(nc) as tc:
        with (
            tc.tile_pool(name="persist", bufs=1) as persist,
            tc.tile_pool(name="acts", bufs=3) as actspool,
            tc.tile_pool(name="tmp", bufs=8) as tpool,
        ):
            whh_sb = persist.tile([128, KC, MC, 128], bf16)
            wp_sb = persist.tile([128, KC, MC, 128], bf16)
            wx_sb = persist.tile([KPAD, MC, 128], bf16)
            c_sb = persist.tile([128, KC, BL], f32)
            hbfA = persist.tile([128, KC, SX, BL], bf16)
            hbfB = persist.tile([128, KC, SX, BL], bf16)
            xrA = persist.tile([KPAD, SX * BL], bf16)
            xrB = persist.tile([KPAD, SX * BL], bf16)

            # gate PSUM per band-set: [16 slots, 16 steps, 8 batch] f32 =
            # 8KB = 4 banks (slot quads f/g/i/o land on bank boundaries).
            # The same-address alias lets sweep matmuls skip dep tracking;
            # each gate group's last matmul writes the tracked tensor and
            # its auto-dep gates that group's sigmoid.
            PS = [nc.place_psum_tensor(f"p{ss}", [128, MC, BSTEP, BL],
                                       f32, 4 * ss) for ss in range(2)]
            PA = [nc.place_psum_tensor(f"pa{ss}", [128, MC, BSTEP, BL],
                                       f32, 4 * ss) for ss in range(2)]

            nc.sync.dma_start(out=whh_sb[:], in_=whh_d[:])
            nc.sync.dma_start(out=wp_sb[:], in_=wp_d[:])
            nc.sync.dma_start(out=wx_sb[:], in_=wx_d[:])
            nc.vector.memset(c_sb[:], 0.0)
            nc.vector.memset(hbfA[:], 0.0)
            nc.vector.memset(hbfB[:], 0.0)

            # pull the sigmoid/tanh ACT table load out of the row loop
            warm = persist.tile([1, 1], f32)
            nc.vector.memset(warm[:], 0.0)
            nc.scalar.activation(out=warm[:], in_=warm[:], func=SIG)
            nc.scalar.activation(out=warm[:], in_=warm[:], func=TANH)

            def pre_slot(s, band, band_set, xrt, hsrc):
                # bias + x @ Wx.T + prevrow @ Wp.T for 16 steps of one slot.
                # start=True only on the first slot of each 2KB PSUM bank
                # (the start flag invalidates the whole bank region).
                dst = PS[band_set][:, s, :, :]
                nc.tensor.matmul(
                    dst, wx_sb[:, s, :],
                    xrt[:, band * BSTEP * BL:(band + 1) * BSTEP * BL],
                    start=s % 4 == 0, stop=False, skip_group_check=True)
                for k in range(KC):
                    nc.tensor.matmul(
                        dst, wp_sb[:, k, s, :],
                        hsrc[:, k, band * BSTEP:(band + 1) * BSTEP, :],
                        start=False, stop=False, skip_group_check=True)

            ts_clock = [1.0]

            def emit_step(j, cur, prev, xr_cur, xr_nxt):
                band_set = (j // BSTEP) % 2
                jl = j % BSTEP
                ts = ts_clock[0]
                ts_clock[0] += 0.01

                def rhs_h(k):
                    if j == 0:
                        return prev[:, k, SX - 1, :]
                    return cur[:, k, j - 1, :]

                def sweep(s0, s1):
                    # only the group's last matmul writes the tracked
                    # tensor; PE executes in order so its dep implies all
                    for s in range(s0, s1):
                        for k in range(KC):
                            last = s == s1 - 1 and k == KC - 1
                            dst = (PS[band_set][:, s, jl, :] if last
                                   else PA[band_set][:, s, jl, :])
                            nc.tensor.matmul(
                                dst, whh_sb[:, k, s, :], rhs_h(k),
                                start=False, stop=(k == KC - 1),
                                skip_group_check=True)

                with tc.tile_wait_until(ts):
                    sweep(0, 4)      # f
                with tc.tile_wait_until(ts + 0.001):
                    acts_f = actspool.tile([128, 4, BL], f32)
                    nc.scalar.activation(
                        out=acts_f[:], in_=PS[band_set][:, 0:4, jl, :],
                        func=SIG)
                with tc.tile_wait_until(ts + 0.002):
                    fc = tpool.tile([128, KC, BL], f32)
                    nc.vector.tensor_mul(fc[:], acts_f[:], c_sb[:])
                with tc.tile_wait_until(ts):
                    sweep(4, 12)     # g, i
                with tc.tile_wait_until(ts + 0.003):
                    acts_gi = actspool.tile([128, 8, BL], f32)
                    nc.scalar.activation(
                        out=acts_gi[:], in_=PS[band_set][:, 4:12, jl, :],
                        func=SIG)
                with tc.tile_wait_until(ts + 0.004):
                    # t2 = (sig(2g) - 0.5) * sig(i) = tanh(g)/2 * sig(i)
                    t2 = tpool.tile([128, KC, BL], f32)
                    nc.vector.scalar_tensor_tensor(
                        out=t2[:], in0=acts_gi[:, 0:4, :], scalar=0.5,
                        in1=acts_gi[:, 4:8, :], op0=SUB, op1=MULT)
                    # c = 2*t2 + f*c
                    nc.vector.scalar_tensor_tensor(
                        out=c_sb[:], in0=t2[:], scalar=2.0, in1=fc[:],
                        op0=MULT, op1=ADD)
                with tc.tile_wait_until(ts):
                    sweep(12, 16)    # o
                with tc.tile_wait_until(ts + 0.005):
                    acts_o = actspool.tile([128, 4, BL], f32)
                    nc.scalar.activation(
                        out=acts_o[:], in_=PS[band_set][:, 12:16, jl, :],
                        func=SIG)
                with tc.tile_wait_until(ts + 0.006):
                    tc_t = tpool.tile([128, KC, BL], f32)
                    nc.scalar.activation(out=tc_t[:], in_=c_sb[:], func=TANH)
                with tc.tile_wait_until(ts + 0.007):
                    nc.vector.tensor_mul(cur[:, :, j, :], acts_o[:],
                                         tc_t[:])

                # PRE for the next band, one slot per step (skip jl=0: its
                # WAR on the band-set may not have cleared yet)
                band_next = j // BSTEP + 1
                if jl == 0:
                    return
                slots = [jl - 1] + ([15] if jl == BSTEP - 1 else [])
                with tc.tile_wait_until(ts + 0.005):
                    for s in slots:
                        if band_next < NBAND:
                            pre_slot(s, band_next, band_next % 2, xr_cur,
                                     prev)
                        else:  # next row's band 0 (this row's h cols 0..15)
                            pre_slot(s, 0, 0, xr_nxt, cur)

            def row_section(cur, prev, xr_cur, xr_nxt, row_expr):
                # prefetch next row's x (row sy reads the zero padding)
                with tc.tile_wait_until(ts_clock[0]):
                    nc.gpsimd.dma_start(
                        out=xr_nxt[:],
                        in_=x_d[:, bass.ds(row_expr + 1, 1), :])
                for j in range(SX):
                    emit_step(j, cur, prev, xr_cur, xr_nxt)
                with tc.tile_wait_until(ts_clock[0]):
                    nc.gpsimd.dma_start(
                        out=out_d[:, :, bass.ds(row_expr * SX, SX), :],
                        in_=cur[:])

            # row 0: x + band 0
            nc.gpsimd.dma_start(out=xrA[:], in_=x_d[:, 0, :])
            for s in range(MC):
                pre_slot(s, 0, 0, xrA, hbfB)

            with tc.For_i(0, sy // UNROLL) as iv:
                for rr in range(UNROLL):
                    if rr % 2 == 0:
                        row_section(hbfA, hbfB, xrA, xrB, iv * UNROLL + rr)
                    else:
                        row_section(hbfB, hbfA, xrB, xrA, iv * UNROLL + rr)

    nc.compile()
    return nc


_CACHE = {}


def _get_module(sy=SY):
    if sy not in _CACHE:
        _CACHE[sy] = _build_module(sy)
    return _CACHE[sy]


def _prep_shared(W_ih, W_hh, b_ih, b_hh):
    perm = np.array(SLOT_TO_MCHUNK)
    scale = np.ones((16, 1), np.float32)
    scale[8:12] = 2.0  # g-gate rows pre-scaled: tanh(g) = 2*sig(2g) - 1

    wih_t = np.ascontiguousarray(W_ih.T.astype(np.float32))     # (560, 2048)
    wih_t = (wih_t.reshape(560, 16, 128) * scale[None]).astype(np.float32)
    wih_t = wih_t[:, perm, :]                                   # slot order
    bias = ((b_ih + b_hh).astype(np.float32).reshape(16, 128) * scale)[perm]
    wx = np.zeros((KPAD, 16, 128), np.float32)
    wx[:IN] = wih_t[:IN]
    wx[IN] = bias
    wx = wx.reshape(KPAD, MC * 128)
    wp = wih_t[IN:]                                             # (512,16,128)
    wp = wp.reshape(KC, 128, MC, 128).transpose(1, 0, 2, 3)
    wp = wp.reshape(128, KC * MC * 128)
    whh = np.ascontiguousarray(W_hh.T.astype(np.float32))       # (512, 2048)
    whh = (whh.reshape(512, 16, 128) * scale[None])[:, perm, :]
    whh = whh.reshape(KC, 128, MC, 128).transpose(1, 0, 2, 3)
    whh = whh.reshape(128, KC * MC * 128)
    return wx.astype(BF16), wp.astype(BF16), whh.astype(BF16)


def _prep_x(batch, sy=SY):
    # xs[i, j, b, :] = patch (C,P,P) flattened, matching the reference
    xs = batch.reshape(B, C, sy, P, SX, P).transpose(2, 4, 0, 1, 3, 5)
    xs = xs.reshape(sy, SX, B, IN)
    per_core = []
    for c in range(NCORES):
        xc = xs[:, :, c * BL:(c + 1) * BL, :]          # (sy, SX, BL, IN)
        xc = xc.transpose(3, 0, 1, 2).reshape(IN, sy, SX * BL)
        xa = np.zeros((KPAD, sy + 1, SX * BL), np.float32)
        xa[:IN, :sy] = xc
        xa[IN, :, :] = 1.0                             # bias ones-row
        per_core.append(xa.astype(BF16))
    return per_core


def _run(batch, W_ih, W_hh, b_ih, b_hh, trace=False):
    from concourse.bass_utils import run_bass_kernel_spmd

    batch = np.asarray(batch, dtype=np.float32)
    wx, wp, whh = _prep_shared(
        np.asarray(W_ih), np.asarray(W_hh), np.asarray(b_ih), np.asarray(b_hh))
    xs = _prep_x(batch)

    nc = _get_module()
    in_maps = [
        {"xt": xs[c], "whht": whh, "wpt": wp, "wxt": wx}
        for c in range(NCORES)
    ]
    res = run_bass_kernel_spmd(nc, in_maps, list(range(NCORES)), trace=trace)

    outs = []
    for c in range(NCORES):
        arr = res.results[c]["out"].astype(np.float32)  # (128, KC, T, BL)
        # reference's to_image is a raw reshape of (B, T, NC) into
        # (B, NC, SY, SX): arr axes (BL, T, KC, 128) flatten to (BL, T*NC).
        arr = arr.transpose(3, 2, 1, 0).reshape(BL, NCELL, SY, SX)
        outs.append(arr)
    return np.concatenate(outs, axis=0).astype(np.float32), res


def kernel(batch, W_ih, W_hh, b_ih, b_hh):
    out, _ = _run(batch, W_ih, W_hh, b_ih, b_hh)
    return out


# revision 12
# speedup vs baseline: 2.0967x; 1.1581x over previous
"""Trainium2 Bass kernel for the 2D-LSTM (nn_Lstm2D) problem.

Reference computation (B=64, C=3, H=W=128, P=4 patch, NC=512 cells):
  - image is cut into a 32x32 grid of 4x4 patches, raster-scanned (1024 steps)
  - per step t=(i,j):  gates = [x_t, h_prevrow_j] @ W_ih.T + h_{t-1} @ W_hh.T + b
                       i,f,g,o = split(gates); c = sig(f)*c + sig(i)*tanh(g)
                       h = sig(o)*tanh(c)
  - output: h at every grid cell -> (B, 512, 32, 32)

Strategy (8 NeuronCores, data-parallel over batch, 8 batch elements/core):
  - The recurrence is a strict 1024-step serial chain; per step the PE runs
    64 weight-tile matmuls (16 gate-chunks x 4 k-chunks, ~34ns each, weight
    load bound) and the sigmoid/tanh/vector tail trails the sweep.
  - per-row "PRE" (bias + x@Wx.T + prevrow@Wp.T) accumulates in PSUM over
    2 bands of 16 steps (N=128 moving -> only 5 PRE matmuls per step); the
    bias rides as a ones-row of x.  Recurrence matmuls accumulate on top
    (start=False); start=True only on the first slot of each 2KB bank.
  - gate PSUM per band-set: [16 slots, 16 steps, 8] f32 = 4 banks; slots
    f 0-3 / g 4-7 / i 8-11 / o 12-15, so sig(f) and sig(gi) fire mid-sweep
    and only sig(o)/tanh/h-mul trail the last matmul.  g-gate rows are
    pre-scaled x2 host-side: tanh(g) = 2*sig(2g)-1.
  - sweep matmuls write a same-address PSUM alias (no tracked deps) except
    each gate group's last, whose auto-dep gates that group's sigmoid.
  - the static scheduler mis-times matmuls and would reorder the ACT/DVE
    streams and bunch PRE matmuls; tile_wait_until sim-time slots pin the
    per-engine stream order (PE: sweep then PRE chunk each step; ACT:
    sig_f, sig_gi, sig_o, tanh; DVE: fc, t2, c, h).
"""

import numpy as np
import ml_dtypes

B = 64
C = 3
H = W = 128
P = 4
NCELL = 512
IN = C * P * P           # 48
IN1 = IN + 1             # +1 ones-row carrying the bias
KPAD = 128               # x contraction zero-padded to full partition dim
SY = SX = 32
NCORES = 8
BL = B // NCORES         # 8 batch elements per core
KC = NCELL // 128        # 4 contraction chunks for h
MC = (4 * NCELL) // 128  # 16 gate-dim chunks
NBAND = 2                # 2 bands of 16 steps per row
BSTEP = SX // NBAND      # 16 steps per band
UNROLL = 8               # rows per hardware-loop iteration
# slot order (m-chunks of 128 gate rows): f0..f3, g0..g3, i0..i3, o0..o3
SLOT_TO_MCHUNK = [4, 5, 6, 7, 8, 9, 10, 11, 0, 1, 2, 3, 12, 13, 14, 15]

BF16 = ml_dtypes.bfloat16


def _build_module(sy=SY):
    import concourse.bass as bass
    import concourse.bacc as bacc
    import concourse.tile as tile
    import concourse.mybir as mybir

    f32 = mybir.dt.float32
    bf16 = mybir.dt.bfloat16
    SIG = mybir.ActivationFunctionType.Sigmoid
    TANH = mybir.ActivationFunctionType.Tanh
    SUB = mybir.AluOpType.subtract
    MULT = mybir.AluOpType.mult
    ADD = mybir.AluOpType.add

    nc = bacc.Bacc()

    # x rows padded by one (prefetch of row sy reads harmless zeros)
    x_d = nc.declare_dram_parameter("xt", [KPAD, sy + 1, SX * BL], bf16,
                                    isOutput=False)
    whh_d = nc.declare_dram_parameter("whht", [128, KC * MC * 128], bf16,
                                      isOutput=False)
    wp_d = nc.declare_dram_parameter("wpt", [128, KC * MC * 128], bf16,
                                     isOutput=False)
    wx_d = nc.declare_dram_parameter("wxt", [KPAD, MC * 128], bf16,
                                     isOutput=False)
    out_d = nc.declare_dram_parameter("out", [128, KC, sy * SX, BL], bf16,
                                      isOutput=True)

    with tile.TileContext(nc) as tc:
        with (
            tc.tile_pool(name="persist", bufs=1) as persist,
            tc.tile_pool(name="acts", bufs=3) as actspool,
            tc.tile_pool(name="tmp", bufs=8) as tpool,
        ):
            whh_sb = persist.tile([128, KC, MC, 128], bf16)
            wp_sb = persist.tile([128, KC, MC, 128], bf16)
            wx_sb = persist.tile([KPAD, MC, 128], bf16)
            c_sb = persist.tile([128, KC, BL], f32)
            hbfA = persist.tile([128, KC, SX, BL], bf16)
            hbfB = persist.tile([128, KC, SX, BL], bf16)
            xrA = persist.tile([KPAD, SX * BL], bf16)
            xrB = persist.tile([KPAD, SX * BL], bf16)

            # gate PSUM per band-set: separate tensors per gate group so a
            # group's tracked last matmul never picks up a false WAR against
            # the other groups' sigmoids (tracking is tile-granular).  Per
            # set: f 2KB (1 bank) + gi 4KB (2 banks) + o 2KB (1 bank).
            # The same-address alias lets the other sweep matmuls skip dep
            # tracking entirely; PE executes in order so each group's last
            # tracked matmul implies the whole group.
            def pl(name, nslot, bank):
                return nc.place_psum_tensor(
                    name, [128, nslot, BSTEP, BL], f32, bank)

            PF = [pl(f"pf{ss}", 4, 4 * ss) for ss in range(2)]
            PFA = [pl(f"pfa{ss}", 4, 4 * ss) for ss in range(2)]
            PGI = [pl(f"pgi{ss}", 8, 4 * ss + 1) for ss in range(2)]
            PGIA = [pl(f"pgia{ss}", 8, 4 * ss + 1) for ss in range(2)]
            PO = [pl(f"po{ss}", 4, 4 * ss + 3) for ss in range(2)]
            POA = [pl(f"poa{ss}", 4, 4 * ss + 3) for ss in range(2)]

            def psum_slot(s, band_set, jl=None, alias=False):
                if s < 4:
                    t = (PFA if alias else PF)[band_set]
                    sl = s
                elif s < 12:
                    t = (PGIA if alias else PGI)[band_set]
                    sl = s - 4
                else:
                    t = (POA if alias else PO)[band_set]
                    sl = s - 12
                if jl is None:
                    return t[:, sl, :, :]
                return t[:, sl, jl, :]

            nc.sync.dma_start(out=whh_sb[:], in_=whh_d[:])
            nc.sync.dma_start(out=wp_sb[:], in_=wp_d[:])
            nc.sync.dma_start(out=wx_sb[:], in_=wx_d[:])
            nc.vector.memset(c_sb[:], 0.0)
            nc.vector.memset(hbfA[:], 0.0)
            nc.vector.memset(hbfB[:], 0.0)

            # pull the sigmoid/tanh ACT table load out of the row loop
            warm = persist.tile([1, 1], f32)
            nc.vector.memset(warm[:], 0.0)
            nc.scalar.activation(out=warm[:], in_=warm[:], func=SIG)
            nc.scalar.activation(out=warm[:], in_=warm[:], func=TANH)

            def pre_slot(s, band, band_set, xrt, hsrc):
                # bias + x @ Wx.T + prevrow @ Wp.T for 16 steps of one slot.
                # start=True only on the first slot of each 2KB PSUM bank
                # (the start flag invalidates the whole bank region).
                dst = psum_slot(s, band_set)
                nc.tensor.matmul(
                    dst, wx_sb[:, s, :],
                    xrt[:, band * BSTEP * BL:(band + 1) * BSTEP * BL],
                    start=s % 4 == 0, stop=False, skip_group_check=True)
                for k in range(KC):
                    nc.tensor.matmul(
                        dst, wp_sb[:, k, s, :],
                        hsrc[:, k, band * BSTEP:(band + 1) * BSTEP, :],
                        start=False, stop=False, skip_group_check=True)

            ts_clock = [1.0]

            def emit_step(j, cur, prev, xr_cur, xr_nxt):
                band_set = (j // BSTEP) % 2
                jl = j % BSTEP
                ts = ts_clock[0]
                ts_clock[0] += 0.01

                def rhs_h(k):
                    if j == 0:
                        return prev[:, k, SX - 1, :]
                    return cur[:, k, j - 1, :]

                def sweep(s0, s1):
                    # only the group's last matmul writes the tracked
                    # tensor; PE executes in order so its dep implies all
                    for s in range(s0, s1):
                        for k in range(KC):
                            last = s == s1 - 1 and k == KC - 1
                            dst = psum_slot(s, band_set, jl,
                                            alias=not last)
                            nc.tensor.matmul(
                                dst, whh_sb[:, k, s, :], rhs_h(k),
                                start=False, stop=(k == KC - 1),
                                skip_group_check=True)

                with tc.tile_wait_until(ts):
                    sweep(0, 4)      # f
                with tc.tile_wait_until(ts + 0.001):
                    acts_f = actspool.tile([128, 4, BL], f32)
                    nc.scalar.activation(
                        out=acts_f[:], in_=PF[band_set][:, :, jl, :],
                        func=SIG)
                with tc.tile_wait_until(ts + 0.002):
                    fc = tpool.tile([128, KC, BL], f32)
                    nc.vector.tensor_mul(fc[:], acts_f[:], c_sb[:])
                with tc.tile_wait_until(ts):
                    sweep(4, 12)     # g, i
                with tc.tile_wait_until(ts + 0.003):
                    acts_gi = actspool.tile([128, 8, BL], f32)
                    nc.scalar.activation(
                        out=acts_gi[:], in_=PGI[band_set][:, :, jl, :],
                        func=SIG)
                with tc.tile_wait_until(ts + 0.004):
                    # t2 = (sig(2g) - 0.5) * sig(i) = tanh(g)/2 * sig(i)
                    t2 = tpool.tile([128, KC, BL], f32)
                    nc.vector.scalar_tensor_tensor(
                        out=t2[:], in0=acts_gi[:, 0:4, :], scalar=0.5,
                        in1=acts_gi[:, 4:8, :], op0=SUB, op1=MULT)
                    # c = 2*t2 + f*c
                    nc.vector.scalar_tensor_tensor(
                        out=c_sb[:], in0=t2[:], scalar=2.0, in1=fc[:],
                        op0=MULT, op1=ADD)
                with tc.tile_wait_until(ts):
                    sweep(12, 16)    # o
                with tc.tile_wait_until(ts + 0.005):
                    acts_o = actspool.tile([128, 4, BL], f32)
                    nc.scalar.activation(
                        out=acts_o[:], in_=PO[band_set][:, :, jl, :],
                        func=SIG)
                with tc.tile_wait_until(ts + 0.006):
                    tc_t = tpool.tile([128, KC, BL], f32)
                    nc.scalar.activation(out=tc_t[:], in_=c_sb[:], func=TANH)
                with tc.tile_wait_until(ts + 0.007):
                    nc.vector.tensor_mul(cur[:, :, j, :], acts_o[:],
                                         tc_t[:])

                # PRE for the next band, one slot per step (skip jl=0: its
                # WAR on the band-set may not have cleared yet)
                band_next = j // BSTEP + 1
                if jl == 0:
                    return
                slots = [jl - 1] + ([15] if jl == BSTEP - 1 else [])
                with tc.tile_wait_until(ts + 0.005):
                    for s in slots:
                        if band_next < NBAND:
                            pre_slot(s, band_next, band_next % 2, xr_cur,
                                     prev)
                        else:  # next row's band 0 (this row's h cols 0..15)
                            pre_slot(s, 0, 0, xr_nxt, cur)

            def row_section(cur, prev, xr_cur, xr_nxt, row_expr):
                # prefetch next row's x (row sy reads the zero padding)
                with tc.tile_wait_until(ts_clock[0]):
                    nc.gpsimd.dma_start(
                        out=xr_nxt[:],
                        in_=x_d[:, bass.ds(row_expr + 1, 1), :])
                for j in range(SX):
                    emit_step(j, cur, prev, xr_cur, xr_nxt)
                with tc.tile_wait_until(ts_clock[0]):
                    nc.gpsimd.dma_start(
                        out=out_d[:, :, bass.ds(row_expr * SX, SX), :],
                        in_=cur[:])

            # row 0: x + band 0
            nc.gpsimd.dma_start(out=xrA[:], in_=x_d[:, 0, :])
            for s in range(MC):
                pre_slot(s, 0, 0, xrA, hbfB)

            with tc.For_i(0, sy // UNROLL) as iv:
                for rr in range(UNROLL):
                    if rr % 2 == 0:
                        row_section(hbfA, hbfB, xrA, xrB, iv * UNROLL + rr)
                    else:
                        row_section(hbfB, hbfA, xrB, xrA, iv * UNROLL + rr)

    nc.compile()
    return nc


_CACHE = {}


def _get_module(sy=SY):
    if sy not in _CACHE:
        _CACHE[sy] = _build_module(sy)
    return _CACHE[sy]


def _prep_shared(W_ih, W_hh, b_ih, b_hh):
    perm = np.array(SLOT_TO_MCHUNK)
    scale = np.ones((16, 1), np.float32)
    scale[8:12] = 2.0  # g-gate rows pre-scaled: tanh(g) = 2*sig(2g) - 1

    wih_t = np.ascontiguousarray(W_ih.T.astype(np.float32))     # (560, 2048)
    wih_t = (wih_t.reshape(560, 16, 128) * scale[None]).astype(np.float32)
    wih_t = wih_t[:, perm, :]                                   # slot order
    bias = ((b_ih + b_hh).astype(np.float32).reshape(16, 128) * scale)[perm]
    wx = np.zeros((KPAD, 16, 128), np.float32)
    wx[:IN] = wih_t[:IN]
    wx[IN] = bias
    wx = wx.reshape(KPAD, MC * 128)
    wp = wih_t[IN:]                                             # (512,16,128)
    wp = wp.reshape(KC, 128, MC, 128).transpose(1, 0, 2, 3)
    wp = wp.reshape(128, KC * MC * 128)
    whh = np.ascontiguousarray(W_hh.T.astype(np.float32))       # (512, 2048)
    whh = (whh.reshape(512, 16, 128) * scale[None])[:, perm, :]
    whh = whh.reshape(KC, 128, MC, 128).transpose(1, 0, 2, 3)
    whh = whh.reshape(128, KC * MC * 128)
    return wx.astype(BF16), wp.astype(BF16), whh.astype(BF16)


def _prep_x(batch, sy=SY):
    # xs[i, j, b, :] = patch (C,P,P) flattened, matching the reference
    xs = batch.reshape(B, C, sy, P, SX, P).transpose(2, 4, 0, 1, 3, 5)
    xs = xs.reshape(sy, SX, B, IN)
    per_core = []
    for c in range(NCORES):
        xc = xs[:, :, c * BL:(c + 1) * BL, :]          # (sy, SX, BL, IN)
        xc = xc.transpose(3, 0, 1, 2).reshape(IN, sy, SX * BL)
        xa = np.zeros((KPAD, sy + 1, SX * BL), np.float32)
        xa[:IN, :sy] = xc
        xa[IN, :, :] = 1.0                             # bias ones-row
        per_core.append(xa.astype(BF16))
    return per_core


def _run(batch, W_ih, W_hh, b_ih, b_hh, trace=False):
    from concourse.bass_utils import run_bass_kernel_spmd

    batch = np.asarray(batch, dtype=np.float32)
    wx, wp, whh = _prep_shared(
        np.asarray(W_ih), np.asarray(W_hh), np.asarray(b_ih), np.asarray(b_hh))
    xs = _prep_x(batch)

    nc = _get_module()
    in_maps = [
        {"xt": xs[c], "whht": whh, "wpt": wp, "wxt": wx}
        for c in range(NCORES)
    ]
    res = run_bass_kernel_spmd(nc, in_maps, list(range(NCORES)), trace=trace)

    outs = []
    for c in range(NCORES):
        arr = res.results[c]["out"].astype(np.float32)  # (128, KC, T, BL)
        # reference's to_image is a raw reshape of (B, T, NC) into
        # (B, NC, SY, SX): arr axes (BL, T, KC, 128) flatten to (BL, T*NC).
        arr = arr.transpose(3, 2, 1, 0).reshape(BL, NCELL, SY, SX)
        outs.append(arr)
    return np.concatenate(outs, axis=0).astype(np.float32), res


def kernel(batch, W_ih, W_hh, b_ih, b_hh):
    out, _ = _run(batch, W_ih, W_hh, b_ih, b_hh)
    return out


# revision 16
# speedup vs baseline: 2.1171x; 1.0097x over previous
"""Trainium2 Bass kernel for the 2D-LSTM (nn_Lstm2D) problem.

Reference computation (B=64, C=3, H=W=128, P=4 patch, NC=512 cells):
  - image is cut into a 32x32 grid of 4x4 patches, raster-scanned (1024 steps)
  - per step t=(i,j):  gates = [x_t, h_prevrow_j] @ W_ih.T + h_{t-1} @ W_hh.T + b
                       i,f,g,o = split(gates); c = sig(f)*c + sig(i)*tanh(g)
                       h = sig(o)*tanh(c)
  - output: h at every grid cell -> (B, 512, 32, 32)

Strategy (8 NeuronCores, data-parallel over batch, 8 batch elements/core):
  - The recurrence is a strict 1024-step serial chain; per step the PE runs
    64 weight-tile matmuls (16 gate-chunks x 4 k-chunks, ~34ns each, weight
    load bound) and the sigmoid/tanh/vector tail trails the sweep.
  - per-row "PRE" (bias + x@Wx.T + prevrow@Wp.T) accumulates in PSUM over
    2 bands of 16 steps (N=128 moving -> only 5 PRE matmuls per step); the
    bias rides as a ones-row of x.  Recurrence matmuls accumulate on top
    (start=False); start=True only on the first slot of each 2KB bank.
  - gate PSUM per band-set: [16 slots, 16 steps, 8] f32 = 4 banks; slots
    f 0-3 / g 4-7 / i 8-11 / o 12-15, so sig(f) and sig(gi) fire mid-sweep
    and only sig(o)/tanh/h-mul trail the last matmul.  g-gate rows are
    pre-scaled x2 host-side: tanh(g) = 2*sig(2g)-1.
  - sweep matmuls write a same-address PSUM alias (no tracked deps) except
    each gate group's last, whose auto-dep gates that group's sigmoid.
  - the static scheduler mis-times matmuls and would reorder the ACT/DVE
    streams and bunch PRE matmuls; tile_wait_until sim-time slots pin the
    per-engine stream order (PE: sweep then PRE chunk each step; ACT:
    sig_f, sig_gi, sig_o, tanh; DVE: fc, t2, c, h).
"""

import numpy as np
import ml_dtypes

B = 64
C = 3
H = W = 128
P = 4
NCELL = 512
IN = C * P * P           # 48
IN1 = IN + 1             # +1 ones-row carrying the bias
KPAD = 128               # x contraction zero-padded to full partition dim
SY = SX = 32
NCORES = 8
BL = B // NCORES         # 8 batch elements per core
KC = NCELL // 128        # 4 contraction chunks for h
MC = (4 * NCELL) // 128  # 16 gate-dim chunks
NBAND = 2                # 2 bands of 16 steps per row
BSTEP = SX // NBAND      # 16 steps per band
UNROLL = 16              # rows per hardware-loop iteration
# slot order (m-chunks of 128 gate rows): f0..f3, g0..g3, i0..i3, o0..o3
SLOT_TO_MCHUNK = [4, 5, 6, 7, 8, 9, 10, 11, 0, 1, 2, 3, 12, 13, 14, 15]

BF16 = ml_dtypes.bfloat16


def _build_module(sy=SY):
    import concourse.bass as bass
    import concourse.bacc as bacc
    import concourse.tile as tile
    import concourse.mybir as mybir

    f32 = mybir.dt.float32
    bf16 = mybir.dt.bfloat16
    SIG = mybir.ActivationFunctionType.Sigmoid
    TANH = mybir.ActivationFunctionType.Tanh
    SUB = mybir.AluOpType.subtract
    MULT = mybir.AluOpType.mult
    ADD = mybir.AluOpType.add

    nc = bacc.Bacc()

    # x rows padded by one (prefetch of row sy reads harmless zeros)
    x_d = nc.declare_dram_parameter("xt", [KPAD, sy + 1, SX * BL], bf16,
                                    isOutput=False)
    whh_d = nc.declare_dram_parameter("whht", [128, KC * MC * 128], bf16,
                                      isOutput=False)
    wp_d = nc.declare_dram_parameter("wpt", [128, KC * MC * 128], bf16,
                                     isOutput=False)
    wx_d = nc.declare_dram_parameter("wxt", [KPAD, MC * 128], bf16,
                                     isOutput=False)
    out_d = nc.declare_dram_parameter("out", [128, KC, sy * SX, BL], bf16,
                                      isOutput=True)

    with tile.TileContext(nc) as tc:
        with (
            tc.tile_pool(name="persist", bufs=1) as persist,
            tc.tile_pool(name="acts", bufs=8) as actspool,
            tc.tile_pool(name="tmp", bufs=8) as tpool,
        ):
            whh_sb = persist.tile([128, KC, MC, 128], bf16)
            wp_sb = persist.tile([128, KC, MC, 128], bf16)
            wx_sb = persist.tile([KPAD, MC, 128], bf16)
            c_sb = persist.tile([128, KC, BL], f32)
            hbfA = persist.tile([128, KC, SX, BL], bf16)
            hbfB = persist.tile([128, KC, SX, BL], bf16)
            xrA = persist.tile([KPAD, SX * BL], bf16)
            xrB = persist.tile([KPAD, SX * BL], bf16)

            # gate PSUM per band-set: separate tensors per gate group (one
            # bank each: f/g/i/o) so a group's tracked last matmul never
            # picks up a false WAR against the other groups' sigmoids
            # (tracking is tile-granular), and each group's sigmoid is
            # gated at the earliest possible conveyor position.  The
            # same-address alias lets the other sweep matmuls skip dep
            # tracking entirely; PE executes in order so each group's last
            # tracked matmul implies the whole group.
            def pl(name, bank):
                return nc.place_psum_tensor(
                    name, [128, 4, BSTEP, BL], f32, bank)

            PG4 = [[pl(f"pg{g}{ss}", 4 * ss + g) for g in range(4)]
                   for ss in range(2)]
            PA4 = [[pl(f"pa{g}{ss}", 4 * ss + g) for g in range(4)]
                   for ss in range(2)]

            def psum_slot(s, band_set, jl=None, alias=False):
                t = (PA4 if alias else PG4)[band_set][s // 4]
                sl = s % 4
                if jl is None:
                    return t[:, sl, :, :]
                return t[:, sl, jl, :]

            nc.sync.dma_start(out=whh_sb[:], in_=whh_d[:])
            nc.sync.dma_start(out=wp_sb[:], in_=wp_d[:])
            nc.sync.dma_start(out=wx_sb[:], in_=wx_d[:])
            nc.vector.memset(c_sb[:], 0.0)
            nc.vector.memset(hbfA[:], 0.0)
            nc.vector.memset(hbfB[:], 0.0)

            # pull the sigmoid/tanh ACT table load out of the row loop
            warm = persist.tile([1, 1], f32)
            nc.vector.memset(warm[:], 0.0)
            nc.scalar.activation(out=warm[:], in_=warm[:], func=SIG)
            nc.scalar.activation(out=warm[:], in_=warm[:], func=TANH)

            def pre_slot(s, band, band_set, xrt, hsrc):
                # bias + x @ Wx.T + prevrow @ Wp.T for 16 steps of one slot.
                # start=True only on the first slot of each 2KB PSUM bank
                # (the start flag invalidates the whole bank region).
                dst = psum_slot(s, band_set)
                nc.tensor.matmul(
                    dst, wx_sb[:, s, :],
                    xrt[:, band * BSTEP * BL:(band + 1) * BSTEP * BL],
                    start=s % 4 == 0, stop=False, skip_group_check=True)
                for k in range(KC):
                    nc.tensor.matmul(
                        dst, wp_sb[:, k, s, :],
                        hsrc[:, k, band * BSTEP:(band + 1) * BSTEP, :],
                        start=False, stop=False, skip_group_check=True)

            ts_clock = [1.0]

            def emit_step(j, cur, prev, xr_cur, xr_nxt):
                band_set = (j // BSTEP) % 2
                jl = j % BSTEP
                ts = ts_clock[0]
                ts_clock[0] += 0.01

                def rhs_h(k):
                    if j == 0:
                        return prev[:, k, SX - 1, :]
                    return cur[:, k, j - 1, :]

                def sweep(s0, s1):
                    # each bank-group's last matmul writes the tracked
                    # tensor; PE executes in order so its dep implies the
                    # whole group
                    for s in range(s0, s1):
                        for k in range(KC):
                            last = s % 4 == 3 and k == KC - 1
                            dst = psum_slot(s, band_set, jl,
                                            alias=not last)
                            nc.tensor.matmul(
                                dst, whh_sb[:, k, s, :], rhs_h(k),
                                start=False, stop=(k == KC - 1),
                                skip_group_check=True)

                with tc.tile_wait_until(ts):
                    sweep(0, 4)      # f
                with tc.tile_wait_until(ts + 0.001):
                    acts_f = actspool.tile([128, 4, BL], f32)
                    nc.scalar.activation(
                        out=acts_f[:], in_=PG4[band_set][0][:, :, jl, :],
                        func=SIG)
                with tc.tile_wait_until(ts + 0.002):
                    fc = tpool.tile([128, KC, BL], f32)
                    nc.vector.tensor_mul(fc[:], acts_f[:], c_sb[:])
                with tc.tile_wait_until(ts):
                    sweep(4, 12)     # g, i
                with tc.tile_wait_until(ts + 0.003):
                    acts_g = actspool.tile([128, 4, BL], f32)
                    nc.scalar.activation(
                        out=acts_g[:], in_=PG4[band_set][1][:, :, jl, :],
                        func=SIG)
                with tc.tile_wait_until(ts + 0.004):
                    acts_i = actspool.tile([128, 4, BL], f32)
                    nc.scalar.activation(
                        out=acts_i[:], in_=PG4[band_set][2][:, :, jl, :],
                        func=SIG)
                with tc.tile_wait_until(ts + 0.005):
                    # t2 = (sig(2g) - 0.5) * sig(i) = tanh(g)/2 * sig(i)
                    t2 = tpool.tile([128, KC, BL], f32)
                    nc.vector.scalar_tensor_tensor(
                        out=t2[:], in0=acts_g[:], scalar=0.5,
                        in1=acts_i[:], op0=SUB, op1=MULT)
                    # c = 2*t2 + f*c
                    nc.vector.scalar_tensor_tensor(
                        out=c_sb[:], in0=t2[:], scalar=2.0, in1=fc[:],
                        op0=MULT, op1=ADD)
                with tc.tile_wait_until(ts):
                    sweep(12, 16)    # o
                with tc.tile_wait_until(ts + 0.006):
                    acts_o = actspool.tile([128, 4, BL], f32)
                    nc.scalar.activation(
                        out=acts_o[:], in_=PG4[band_set][3][:, :, jl, :],
                        func=SIG)
                with tc.tile_wait_until(ts + 0.007):
                    tc_t = tpool.tile([128, KC, BL], f32)
                    nc.scalar.activation(out=tc_t[:], in_=c_sb[:], func=TANH)
                with tc.tile_wait_until(ts + 0.008):
                    nc.vector.tensor_mul(cur[:, :, j, :], acts_o[:],
                                         tc_t[:])

                # PRE for the next band, one slot per step (skip jl=0: its
                # WAR on the band-set may not have cleared yet)
                band_next = j // BSTEP + 1
                if jl == 0:
                    return
                slots = [jl - 1] + ([15] if jl == BSTEP - 1 else [])
                with tc.tile_wait_until(ts + 0.005):
                    for s in slots:
                        if band_next < NBAND:
                            pre_slot(s, band_next, band_next % 2, xr_cur,
                                     prev)
                        else:  # next row's band 0 (this row's h cols 0..15)
                            pre_slot(s, 0, 0, xr_nxt, cur)

            def row_section(cur, prev, xr_cur, xr_nxt, row_expr):
                # prefetch next row's x (row sy reads the zero padding)
                with tc.tile_wait_until(ts_clock[0]):
                    nc.gpsimd.dma_start(
                        out=xr_nxt[:],
                        in_=x_d[:, bass.ds(row_expr + 1, 1), :])
                for j in range(SX):
                    emit_step(j, cur, prev, xr_cur, xr_nxt)
                with tc.tile_wait_until(ts_clock[0]):
                    nc.gpsimd.dma_start(
                        out=out_d[:, :, bass.ds(row_expr * SX, SX), :],
                        in_=cur[:])

            # row 0: x + band 0
            nc.gpsimd.dma_start(out=xrA[:], in_=x_d[:, 0, :])
            for s in range(MC):
                pre_slot(s, 0, 0, xrA, hbfB)

            with tc.For_i(0, sy // UNROLL) as iv:
                for rr in range(UNROLL):
                    if rr % 2 == 0:
                        row_section(hbfA, hbfB, xrA, xrB, iv * UNROLL + rr)
                    else:
                        row_section(hbfB, hbfA, xrB, xrA, iv * UNROLL + rr)

    nc.compile()
    return nc


_CACHE = {}


def _get_module(sy=SY):
    if sy not in _CACHE:
        _CACHE[sy] = _build_module(sy)
    return _CACHE[sy]


def _prep_shared(W_ih, W_hh, b_ih, b_hh):
    perm = np.array(SLOT_TO_MCHUNK)
    scale = np.ones((16, 1), np.float32)
    scale[8:12] = 2.0  # g-gate rows pre-scaled: tanh(g) = 2*sig(2g) - 1

    wih_t = np.ascontiguousarray(W_ih.T.astype(np.float32))     # (560, 2048)
    wih_t = (wih_t.reshape(560, 16, 128) * scale[None]).astype(np.float32)
    wih_t = wih_t[:, perm, :]                                   # slot order
    bias = ((b_ih + b_hh).astype(np.float32).reshape(16, 128) * scale)[perm]
    wx = np.zeros((KPAD, 16, 128), np.float32)
    wx[:IN] = wih_t[:IN]
    wx[IN] = bias
    wx = wx.reshape(KPAD, MC * 128)
    wp = wih_t[IN:]                                             # (512,16,128)
    wp = wp.reshape(KC, 128, MC, 128).transpose(1, 0, 2, 3)
    wp = wp.reshape(128, KC * MC * 128)
    whh = np.ascontiguousarray(W_hh.T.astype(np.float32))       # (512, 2048)
    whh = (whh.reshape(512, 16, 128) * scale[None])[:, perm, :]
    whh = whh.reshape(KC, 128, MC, 128).transpose(1, 0, 2, 3)
    whh = whh.reshape(128, KC * MC * 128)
    return wx.astype(BF16), wp.astype(BF16), whh.astype(BF16)


def _prep_x(batch, sy=SY):
    # xs[i, j, b, :] = patch (C,P,P) flattened, matching the reference
    xs = batch.reshape(B, C, sy, P, SX, P).transpose(2, 4, 0, 1, 3, 5)
    xs = xs.reshape(sy, SX, B, IN)
    per_core = []
    for c in range(NCORES):
        xc = xs[:, :, c * BL:(c + 1) * BL, :]          # (sy, SX, BL, IN)
        xc = xc.transpose(3, 0, 1, 2).reshape(IN, sy, SX * BL)
        xa = np.zeros((KPAD, sy + 1, SX * BL), np.float32)
        xa[:IN, :sy] = xc
        xa[IN, :, :] = 1.0                             # bias ones-row
        per_core.append(xa.astype(BF16))
    return per_core


def _run(batch, W_ih, W_hh, b_ih, b_hh, trace=False):
    from concourse.bass_utils import run_bass_kernel_spmd

    batch = np.asarray(batch, dtype=np.float32)
    wx, wp, whh = _prep_shared(
        np.asarray(W_ih), np.asarray(W_hh), np.asarray(b_ih), np.asarray(b_hh))
    xs = _prep_x(batch)

    nc = _get_module()
    in_maps = [
        {"xt": xs[c], "whht": whh, "wpt": wp, "wxt": wx}
        for c in range(NCORES)
    ]
    res = run_bass_kernel_spmd(nc, in_maps, list(range(NCORES)), trace=trace)

    outs = []
    for c in range(NCORES):
        arr = res.results[c]["out"].astype(np.float32)  # (128, KC, T, BL)
        # reference's to_image is a raw reshape of (B, T, NC) into
        # (B, NC, SY, SX): arr axes (BL, T, KC, 128) flatten to (BL, T*NC).
        arr = arr.transpose(3, 2, 1, 0).reshape(BL, NCELL, SY, SX)
        outs.append(arr)
    return np.concatenate(outs, axis=0).astype(np.float32), res


def kernel(batch, W_ih, W_hh, b_ih, b_hh):
    out, _ = _run(batch, W_ih, W_hh, b_ih, b_hh)
    return out


# revision 17
# speedup vs baseline: 2.1179x; 1.0004x over previous
"""Trainium2 Bass kernel for the 2D-LSTM (nn_Lstm2D) problem.

Reference computation (B=64, C=3, H=W=128, P=4 patch, NC=512 cells):
  - image is cut into a 32x32 grid of 4x4 patches, raster-scanned (1024 steps)
  - per step t=(i,j):  gates = [x_t, h_prevrow_j] @ W_ih.T + h_{t-1} @ W_hh.T + b
                       i,f,g,o = split(gates); c = sig(f)*c + sig(i)*tanh(g)
                       h = sig(o)*tanh(c)
  - output: h at every grid cell -> (B, 512, 32, 32)

Strategy (8 NeuronCores, data-parallel over batch, 8 batch elements/core):
  - The recurrence is a strict 1024-step serial chain; per step the PE runs
    64 weight-tile matmuls (16 gate-chunks x 4 k-chunks, ~34ns each, weight
    load bound) and the sigmoid/tanh/vector tail trails the sweep.
  - per-row "PRE" (bias + x@Wx.T + prevrow@Wp.T) accumulates in PSUM over
    2 bands of 16 steps (N=128 moving -> only 5 PRE matmuls per step); the
    bias rides as a ones-row of x.  Recurrence matmuls accumulate on top
    (start=False); start=True only on the first slot of each 2KB bank.
  - gate PSUM per band-set: [16 slots, 16 steps, 8] f32 = 4 banks; slots
    f 0-3 / g 4-7 / i 8-11 / o 12-15, so sig(f) and sig(gi) fire mid-sweep
    and only sig(o)/tanh/h-mul trail the last matmul.  g-gate rows are
    pre-scaled x2 host-side: tanh(g) = 2*sig(2g)-1.
  - sweep matmuls write a same-address PSUM alias (no tracked deps) except
    each gate group's last, whose auto-dep gates that group's sigmoid.
  - the static scheduler mis-times matmuls and would reorder the ACT/DVE
    streams and bunch PRE matmuls; tile_wait_until sim-time slots pin the
    per-engine stream order (PE: sweep then PRE chunk each step; ACT:
    sig_f, sig_gi, sig_o, tanh; DVE: fc, t2, c, h).
"""

import numpy as np
import ml_dtypes

B = 64
C = 3
H = W = 128
P = 4
NCELL = 512
IN = C * P * P           # 48
IN1 = IN + 1             # +1 ones-row carrying the bias
KPAD = 128               # x contraction zero-padded to full partition dim
SY = SX = 32
NCORES = 8
BL = B // NCORES         # 8 batch elements per core
KC = NCELL // 128        # 4 contraction chunks for h
MC = (4 * NCELL) // 128  # 16 gate-dim chunks
NBAND = 2                # 2 bands of 16 steps per row
BSTEP = SX // NBAND      # 16 steps per band
UNROLL = 16              # rows per hardware-loop iteration
# slot order (m-chunks of 128 gate rows): f0..f3, g0..g3, i0..i3, o0..o3
SLOT_TO_MCHUNK = [4, 5, 6, 7, 8, 9, 10, 11, 0, 1, 2, 3, 12, 13, 14, 15]

BF16 = ml_dtypes.bfloat16


def _build_module(sy=SY):
    import concourse.bass as bass
    import concourse.bacc as bacc
    import concourse.tile as tile
    import concourse.mybir as mybir

    f32 = mybir.dt.float32
    bf16 = mybir.dt.bfloat16
    SIG = mybir.ActivationFunctionType.Sigmoid
    TANH = mybir.ActivationFunctionType.Tanh
    SUB = mybir.AluOpType.subtract
    MULT = mybir.AluOpType.mult
    ADD = mybir.AluOpType.add

    nc = bacc.Bacc()

    # x rows padded by one (prefetch of row sy reads harmless zeros)
    x_d = nc.declare_dram_parameter("xt", [KPAD, sy + 1, SX * BL], bf16,
                                    isOutput=False)
    whh_d = nc.declare_dram_parameter("whht", [128, KC * MC * 128], bf16,
                                      isOutput=False)
    wp_d = nc.declare_dram_parameter("wpt", [128, KC * MC * 128], bf16,
                                     isOutput=False)
    wx_d = nc.declare_dram_parameter("wxt", [KPAD, MC * 128], bf16,
                                     isOutput=False)
    out_d = nc.declare_dram_parameter("out", [128, KC, sy * SX, BL], bf16,
                                      isOutput=True)

    with tile.TileContext(nc) as tc:
        with (
            tc.tile_pool(name="persist", bufs=1) as persist,
            tc.tile_pool(name="acts", bufs=8) as actspool,
            tc.tile_pool(name="tmp", bufs=8) as tpool,
        ):
            whh_sb = persist.tile([128, KC, MC, 128], bf16)
            wp_sb = persist.tile([128, KC, MC, 128], bf16)
            wx_sb = persist.tile([KPAD, MC, 128], bf16)
            c_sb = persist.tile([128, KC, BL], f32)
            hbfA = persist.tile([128, KC, SX, BL], bf16)
            hbfB = persist.tile([128, KC, SX, BL], bf16)
            xrA = persist.tile([KPAD, SX * BL], bf16)
            xrB = persist.tile([KPAD, SX * BL], bf16)

            # gate PSUM per band-set: separate tensors per gate group (one
            # bank each: f/g/i/o) so a group's tracked last matmul never
            # picks up a false WAR against the other groups' sigmoids
            # (tracking is tile-granular), and each group's sigmoid is
            # gated at the earliest possible conveyor position.  The
            # same-address alias lets the other sweep matmuls skip dep
            # tracking entirely; PE executes in order so each group's last
            # tracked matmul implies the whole group.
            def pl(name, bank):
                return nc.place_psum_tensor(
                    name, [128, 4, BSTEP, BL], f32, bank)

            PG4 = [[pl(f"pg{g}{ss}", 4 * ss + g) for g in range(4)]
                   for ss in range(2)]
            PA4 = [[pl(f"pa{g}{ss}", 4 * ss + g) for g in range(4)]
                   for ss in range(2)]

            def psum_slot(s, band_set, jl=None, alias=False):
                t = (PA4 if alias else PG4)[band_set][s // 4]
                sl = s % 4
                if jl is None:
                    return t[:, sl, :, :]
                return t[:, sl, jl, :]

            nc.sync.dma_start(out=whh_sb[:], in_=whh_d[:])
            nc.sync.dma_start(out=wp_sb[:], in_=wp_d[:])
            nc.sync.dma_start(out=wx_sb[:], in_=wx_d[:])
            nc.vector.memset(c_sb[:], 0.0)
            nc.vector.memset(hbfA[:], 0.0)
            nc.vector.memset(hbfB[:], 0.0)

            # pull the sigmoid/tanh ACT table load out of the row loop
            warm = persist.tile([1, 1], f32)
            nc.vector.memset(warm[:], 0.0)
            nc.scalar.activation(out=warm[:], in_=warm[:], func=SIG)
            nc.scalar.activation(out=warm[:], in_=warm[:], func=TANH)

            def pre_slot(s, band, band_set, xrt, hsrc):
                # bias + x @ Wx.T + prevrow @ Wp.T for 16 steps of one slot.
                # start=True only on the first slot of each 2KB PSUM bank
                # (the start flag invalidates the whole bank region).
                dst = psum_slot(s, band_set)
                nc.tensor.matmul(
                    dst, wx_sb[:, s, :],
                    xrt[:, band * BSTEP * BL:(band + 1) * BSTEP * BL],
                    start=s % 4 == 0, stop=False, skip_group_check=True)
                for k in range(KC):
                    nc.tensor.matmul(
                        dst, wp_sb[:, k, s, :],
                        hsrc[:, k, band * BSTEP:(band + 1) * BSTEP, :],
                        start=False, stop=False, skip_group_check=True)

            ts_clock = [1.0]

            def emit_step(j, cur, prev, xr_cur, xr_nxt):
                band_set = (j // BSTEP) % 2
                jl = j % BSTEP
                ts = ts_clock[0]
                ts_clock[0] += 0.01

                def rhs_h(k):
                    if j == 0:
                        return prev[:, k, SX - 1, :]
                    return cur[:, k, j - 1, :]

                def sweep(s0, s1):
                    # each bank-group's last matmul writes the tracked
                    # tensor; PE executes in order so its dep implies the
                    # whole group
                    for s in range(s0, s1):
                        for k in range(KC):
                            last = s % 4 == 3 and k == KC - 1
                            dst = psum_slot(s, band_set, jl,
                                            alias=not last)
                            nc.tensor.matmul(
                                dst, whh_sb[:, k, s, :], rhs_h(k),
                                start=False, stop=(k == KC - 1),
                                skip_group_check=True)

                # sweep order g, i, f, o: sig(g)/sig(i) are gated at
                # conveyor positions 16/32 so t2 computes early; the
                # pos-48 sigmoid lands on the f-path whose fc -> c chain
                # overlaps sig(o)'s pos-64 conveyor gate.
                with tc.tile_wait_until(ts):
                    sweep(4, 12)     # g, i
                with tc.tile_wait_until(ts + 0.001):
                    acts_g = actspool.tile([128, 4, BL], f32)
                    nc.scalar.activation(
                        out=acts_g[:], in_=PG4[band_set][1][:, :, jl, :],
                        func=SIG)
                with tc.tile_wait_until(ts + 0.002):
                    acts_i = actspool.tile([128, 4, BL], f32)
                    nc.scalar.activation(
                        out=acts_i[:], in_=PG4[band_set][2][:, :, jl, :],
                        func=SIG)
                with tc.tile_wait_until(ts + 0.003):
                    # t2 = (sig(2g) - 0.5) * sig(i) = tanh(g)/2 * sig(i)
                    t2 = tpool.tile([128, KC, BL], f32)
                    nc.vector.scalar_tensor_tensor(
                        out=t2[:], in0=acts_g[:], scalar=0.5,
                        in1=acts_i[:], op0=SUB, op1=MULT)
                with tc.tile_wait_until(ts):
                    sweep(0, 4)      # f
                with tc.tile_wait_until(ts + 0.004):
                    acts_f = actspool.tile([128, 4, BL], f32)
                    nc.scalar.activation(
                        out=acts_f[:], in_=PG4[band_set][0][:, :, jl, :],
                        func=SIG)
                with tc.tile_wait_until(ts + 0.005):
                    fc = tpool.tile([128, KC, BL], f32)
                    nc.vector.tensor_mul(fc[:], acts_f[:], c_sb[:])
                    # c = 2*t2 + f*c
                    nc.vector.scalar_tensor_tensor(
                        out=c_sb[:], in0=t2[:], scalar=2.0, in1=fc[:],
                        op0=MULT, op1=ADD)
                with tc.tile_wait_until(ts):
                    sweep(12, 16)    # o
                with tc.tile_wait_until(ts + 0.006):
                    acts_o = actspool.tile([128, 4, BL], f32)
                    nc.scalar.activation(
                        out=acts_o[:], in_=PG4[band_set][3][:, :, jl, :],
                        func=SIG)
                with tc.tile_wait_until(ts + 0.007):
                    tc_t = tpool.tile([128, KC, BL], f32)
                    nc.scalar.activation(out=tc_t[:], in_=c_sb[:], func=TANH)
                with tc.tile_wait_until(ts + 0.008):
                    nc.vector.tensor_mul(cur[:, :, j, :], acts_o[:],
                                         tc_t[:])

                # PRE for the next band, one slot per step (skip jl=0: its
                # WAR on the band-set may not have cleared yet)
                band_next = j // BSTEP + 1
                if jl == 0:
                    return
                slots = [jl - 1] + ([15] if jl == BSTEP - 1 else [])
                with tc.tile_wait_until(ts + 0.005):
                    for s in slots:
                        if band_next < NBAND:
                            pre_slot(s, band_next, band_next % 2, xr_cur,
                                     prev)
                        else:  # next row's band 0 (this row's h cols 0..15)
                            pre_slot(s, 0, 0, xr_nxt, cur)

            def row_section(cur, prev, xr_cur, xr_nxt, row_expr):
                # prefetch next row's x (row sy reads the zero padding)
                with tc.tile_wait_until(ts_clock[0]):
                    nc.gpsimd.dma_start(
                        out=xr_nxt[:],
                        in_=x_d[:, bass.ds(row_expr + 1, 1), :])
                for j in range(SX):
                    emit_step(j, cur, prev, xr_cur, xr_nxt)
                with tc.tile_wait_until(ts_clock[0]):
                    nc.gpsimd.dma_start(
                        out=out_d[:, :, bass.ds(row_expr * SX, SX), :],
                        in_=cur[:])

            # row 0: x + band 0
            nc.gpsimd.dma_start(out=xrA[:], in_=x_d[:, 0, :])
            for s in range(MC):
                pre_slot(s, 0, 0, xrA, hbfB)

            with tc.For_i(0, sy // UNROLL) as iv:
                for rr in range(UNROLL):
                    if rr % 2 == 0:
                        row_section(hbfA, hbfB, xrA, xrB, iv * UNROLL + rr)
                    else:
                        row_section(hbfB, hbfA, xrB, xrA, iv * UNROLL + rr)

    nc.compile()
    return nc


_CACHE = {}


def _get_module(sy=SY):
    if sy not in _CACHE:
        _CACHE[sy] = _build_module(sy)
    return _CACHE[sy]


def _prep_shared(W_ih, W_hh, b_ih, b_hh):
    perm = np.array(SLOT_TO_MCHUNK)
    scale = np.ones((16, 1), np.float32)
    scale[8:12] = 2.0  # g-gate rows pre-scaled: tanh(g) = 2*sig(2g) - 1

    wih_t = np.ascontiguousarray(W_ih.T.astype(np.float32))     # (560, 2048)
    wih_t = (wih_t.reshape(560, 16, 128) * scale[None]).astype(np.float32)
    wih_t = wih_t[:, perm, :]                                   # slot order
    bias = ((b_ih + b_hh).astype(np.float32).reshape(16, 128) * scale)[perm]
    wx = np.zeros((KPAD, 16, 128), np.float32)
    wx[:IN] = wih_t[:IN]
    wx[IN] = bias
    wx = wx.reshape(KPAD, MC * 128)
    wp = wih_t[IN:]                                             # (512,16,128)
    wp = wp.reshape(KC, 128, MC, 128).transpose(1, 0, 2, 3)
    wp = wp.reshape(128, KC * MC * 128)
    whh = np.ascontiguousarray(W_hh.T.astype(np.float32))       # (512, 2048)
    whh = (whh.reshape(512, 16, 128) * scale[None])[:, perm, :]
    whh = whh.reshape(KC, 128, MC, 128).transpose(1, 0, 2, 3)
    whh = whh.reshape(128, KC * MC * 128)
    return wx.astype(BF16), wp.astype(BF16), whh.astype(BF16)


def _prep_x(batch, sy=SY):
    # xs[i, j, b, :] = patch (C,P,P) flattened, matching the reference
    xs = batch.reshape(B, C, sy, P, SX, P).transpose(2, 4, 0, 1, 3, 5)
    xs = xs.reshape(sy, SX, B, IN)
    per_core = []
    for c in range(NCORES):
        xc = xs[:, :, c * BL:(c + 1) * BL, :]          # (sy, SX, BL, IN)
        xc = xc.transpose(3, 0, 1, 2).reshape(IN, sy, SX * BL)
        xa = np.zeros((KPAD, sy + 1, SX * BL), np.float32)
        xa[:IN, :sy] = xc
        xa[IN, :, :] = 1.0                             # bias ones-row
        per_core.append(xa.astype(BF16))
    return per_core


def _run(batch, W_ih, W_hh, b_ih, b_hh, trace=False):
    from concourse.bass_utils import run_bass_kernel_spmd

    batch = np.asarray(batch, dtype=np.float32)
    wx, wp, whh = _prep_shared(
        np.asarray(W_ih), np.asarray(W_hh), np.asarray(b_ih), np.asarray(b_hh))
    xs = _prep_x(batch)

    nc = _get_module()
    in_maps = [
        {"xt": xs[c], "whht": whh, "wpt": wp, "wxt": wx}
        for c in range(NCORES)
    ]
    res = run_bass_kernel_spmd(nc, in_maps, list(range(NCORES)), trace=trace)

    outs = []
    for c in range(NCORES):
        arr = res.results[c]["out"].astype(np.float32)  # (128, KC, T, BL)
        # reference's to_image is a raw reshape of (B, T, NC) into
        # (B, NC, SY, SX): arr axes (BL, T, KC, 128) flatten to (BL, T*NC).
        arr = arr.transpose(3, 2, 1, 0).reshape(BL, NCELL, SY, SX)
        outs.append(arr)
    return np.concatenate(outs, axis=0).astype(np.float32), res


def kernel(batch, W_ih, W_hh, b_ih, b_hh):
    out, _ = _run(batch, W_ih, W_hh, b_ih, b_hh)
    return out
